# revision 57
# baseline (speedup 1.0000x reference)
"""Multi-head attention (B=2, T=2048, D=1024, H=16, causal) on 8 TRN2 NeuronCores.

Sharding (tensor-parallel heads + token-parallel epilogue):
  - Core c owns heads (2c, 2c+1) -> a 128-wide slice of the QKV output dim.
  - QKV projections: qT/kT/vT [128, B*T] feature-major, from a host-re-tiled
    x^T (one contiguous 16KB descriptor per partition per token slab) and
    host-pre-transposed weight slices (float32r matmuls, 1 cyc/row).
  - Attention: streaming over 128-wide key blocks, transposed score tiles
    S^T [k, q] for both heads in one [128, 1024] PSUM pair; causal mask is a
    -300 staircase *bias* accumulated by an identity-stationary matmul (exp
    of masked entries underflows to 0); one wide exp per k-block on ScalarE;
    ctx^T accumulates with an appended ones-column in v so row 64 of the
    accumulator is the softmax denominator.  The k-loop is software-pipelined
    (ctx of block k issues after scores of block k+1).
  - Emission interleaves batch-1 projections into batch-0 attention so the
    TensorE stream stays dense and ScalarE is never the only busy engine.
  - AllToAll over token slices redistributes ctx^T (2 MB/core minimal
    exchange); output projection is token-sharded; host concatenates.
"""

import numpy as np

import concourse.bacc as bacc
import concourse.bass as bass
import concourse.mybir as mybir
import concourse.tile as tile
from concourse import bass_utils
from concourse.bass import ts

D = 1024
H = 16
DK = D // H  # 64
NCORES = 8
HPC = H // NCORES  # heads per core = 2
DSL = HPC * DK  # per-core QKV output slice = 128
P = 128
QBLK = 512
KBLK = 128
DA = DK + 1  # 65: head dim + ones column (softmax denominator row)

F32 = mybir.dt.float32
F32R = mybir.dt.float32r
EXP = mybir.ActivationFunctionType.Exp
IDENT = mybir.ActivationFunctionType.Identity


def build_nc(B=2, T=2048):
    """Build the SPMD Bass module (identical program on all 8 cores)."""
    NTOK = B * T
    TPC = NTOK // NCORES  # tokens per core in the output projection
    KO = D // P  # 8 contraction chunks
    NKB = T // KBLK  # key blocks per batch
    NQB = T // QBLK  # query blocks per batch
    TB = TPC // P  # 128-token sub-blocks in the output projection
    NSLAB = NTOK // QBLK  # x token slabs
    NPAIR = NSLAB // 2

    nc = bacc.Bacc("TRN2", target_bir_lowering=False, debug=False,
                   num_devices=NCORES)

    # ---- DRAM I/O ------------------------------------------------------
    xT_d = nc.dram_tensor("xT", [P, NSLAB, KO, QBLK], F32R, kind="ExternalInput")
    wqT_d = nc.dram_tensor("wqT", [D, DSL], F32R, kind="ExternalInput")
    wkT_d = nc.dram_tensor("wkT", [D, DSL], F32R, kind="ExternalInput")
    wvT_d = nc.dram_tensor("wvT", [D, DSL], F32R, kind="ExternalInput")
    woT_d = nc.dram_tensor("woT", [D, D], F32R, kind="ExternalInput")
    bq_d = nc.dram_tensor("bq", [DSL, 1], F32, kind="ExternalInput")
    bk_d = nc.dram_tensor("bk", [DSL, 1], F32, kind="ExternalInput")
    bv_d = nc.dram_tensor("bv", [DSL, 1], F32, kind="ExternalInput")
    bo_d = nc.dram_tensor("bo", [D], F32, kind="ExternalInput")
    mask_d = nc.dram_tensor("mask", [P, 2 * QBLK - KBLK], F32R,
                            kind="ExternalInput")
    ident_d = nc.dram_tensor("ident", [P, P], F32R, kind="ExternalInput")
    ones_d = nc.dram_tensor("ones", [P, P], F32R, kind="ExternalInput")
    out_d = nc.dram_tensor("out", [TPC, D], F32, kind="ExternalOutput")

    with tile.TileContext(nc) as tc:
        with (
            tc.tile_pool(name="consts", bufs=1) as consts,
            tc.tile_pool(name="acts", bufs=1) as acts,
            tc.tile_pool(name="xin", bufs=3) as xin,
            tc.tile_pool(name="attn", bufs=2) as attn_pool,
            tc.tile_pool(name="small", bufs=1) as small,
            tc.tile_pool(name="outg", bufs=1) as outg,
            tc.tile_pool(name="outp", bufs=1) as outp,
            tc.tile_pool(name="psA", bufs=2, space="PSUM") as psA,
            tc.tile_pool(name="psC", bufs=2, space="PSUM") as psC,
            tc.tile_pool(name="dram", bufs=2, space="DRAM") as dram,
        ):
            # ---- small constants -----------------------------------
            bq_sb = consts.tile([P, 1], F32, tag="bq")
            bk_sb = consts.tile([P, 1], F32, tag="bk")
            bv_sb = consts.tile([P, 1], F32, tag="bv")
            nc.sync.dma_start(bq_sb[:], bq_d.ap())
            nc.sync.dma_start(bk_sb[:], bk_d.ap())
            nc.sync.dma_start(bv_sb[:], bv_d.ap())
            mask_sb = consts.tile([P, 2 * QBLK - KBLK], F32R, tag="mask")
            nc.sync.dma_start(mask_sb[:], mask_d.ap())
            ident_sb = consts.tile([P, P], F32R, tag="ident")
            nc.sync.dma_start(ident_sb[:], ident_d.ap())

            # QKV weights, loaded per-ko chunk so the first matmuls can
            # start after ~64KB instead of the full 1.5MB.
            wq_sb = consts.tile([P, KO, DSL], F32R, tag="wq")
            wk_sb = consts.tile([P, KO, DSL], F32R, tag="wk")
            wv_sb = consts.tile([P, KO, DSL], F32R, tag="wv")
            for ko in range(KO):
                for w_sb, w_d in ((wq_sb, wqT_d), (wk_sb, wkT_d),
                                  (wv_sb, wvT_d)):
                    nc.sync.dma_start(
                        w_sb[:, ko],
                        w_d.ap().rearrange("(ko p) m -> p ko m", p=P)[:, ko])

            qT = acts.tile([P, NTOK], F32R, tag="qT")
            kT = acts.tile([P, NTOK], F32R, tag="kT")
            vT = acts.tile([P, NTOK], F32R, tag="vT")
            v_nat = acts.tile([P, NTOK // P, 2 * DA], F32R, tag="v_nat")
            nc.sync.dma_start(v_nat[:, :, DK], ones_d.ap()[:, 0:NTOK // P])
            nc.sync.dma_start(v_nat[:, :, DA + DK], ones_d.ap()[:, 0:NTOK // P])

            def proj_pair(i):
                """QKV projections for token slabs 2i, 2i+1 (one stationary
                load per (proj, ko), wide PSUM + one wide epilogue ACT)."""
                xt0 = xin.tile([P, KO, QBLK], F32R, tag="xt", name="xt0")
                xt1 = xin.tile([P, KO, QBLK], F32R, tag="xt", name="xt1")
                nc.sync.dma_start(xt0[:], xT_d.ap()[:, 2 * i])
                nc.sync.dma_start(xt1[:], xT_d.ap()[:, 2 * i + 1])
                for w_sb, b_sb, dst in ((wq_sb, bq_sb, qT),
                                        (wk_sb, bk_sb, kT),
                                        (wv_sb, bv_sb, vT)):
                    ps = psA.tile([P, 2 * QBLK], F32, tag="sp", name="ps")
                    for ko in range(KO):
                        nc.tensor.matmul(ps[:, 0:QBLK], w_sb[:, ko],
                                         xt0[:, ko], start=(ko == 0),
                                         stop=(ko == KO - 1))
                        nc.tensor.matmul(ps[:, QBLK:], w_sb[:, ko],
                                         xt1[:, ko], start=(ko == 0),
                                         stop=(ko == KO - 1))
                    nc.scalar.activation(dst[:, ts(i, 2 * QBLK)], ps[:],
                                         IDENT, bias=b_sb[:, 0:1])

            def v_nat_block(j):
                """Transpose one [128,128] vT tile into v_nat (both heads),
                leaving the ones columns intact."""
                ptf = psA.tile([P, 2 * QBLK], F32R, tag="sp", name="ptf")
                pt = ptf[:, :P]
                nc.tensor.transpose(pt[:], vT[:, ts(j, P)], ident_sb[:])
                nc.vector.tensor_copy(v_nat[:, j, 0:DK], pt[:, 0:DK])
                nc.vector.tensor_copy(v_nat[:, j, DA:DA + DK], pt[:, DK:P])

            a2a_in = dram.tile([NCORES, P, TPC], F32R, tag="a2a_in")
            a2a_out = dram.tile([NCORES, P, TPC], F32R, tag="a2a_out")

            def attention_qblock(b, qi):
                q_sl = ts(b * T // QBLK + qi, QBLK)
                nkb = (qi + 1) * (QBLK // KBLK)
                C0 = psC.tile([P, QBLK], F32, tag="ctx0", name="C0")
                C1 = psC.tile([P, QBLK], F32, tag="ctx1", name="C1")

                def emit_ctx(pend):
                    ap_, jjp, st, sp = pend
                    nc.tensor.matmul(C0[0:DA], v_nat[:, jjp, 0:DA],
                                     ap_[:, 0:QBLK], start=st, stop=sp)
                    nc.tensor.matmul(C1[0:DA], v_nat[:, jjp, DA:2 * DA],
                                     ap_[:, QBLK:], start=st, stop=sp)

                pend = None
                for ki in range(nkb):
                    k_sl = ts(b * T // KBLK + ki, KBLK)
                    jj = b * NKB + ki
                    doff = ki * KBLK - qi * QBLK
                    diag = doff >= 0
                    sp_t = psA.tile([P, 2 * QBLK], F32, tag="sp", name="sp_t")
                    nc.tensor.matmul(sp_t[:, 0:QBLK],
                                     kT[0:DK, k_sl], qT[0:DK, q_sl],
                                     start=True, stop=not diag,
                                     tile_position=(0, 0))
                    nc.tensor.matmul(sp_t[:, QBLK:],
                                     kT[DK:P, k_sl], qT[DK:P, q_sl],
                                     start=True, stop=not diag,
                                     tile_position=(64, 0))
                    if diag:
                        # causal staircase bias (-300 where masked)
                        s = QBLK - KBLK - doff
                        m = mask_sb[:, s:s + QBLK]
                        nc.tensor.matmul(sp_t[:, 0:QBLK], ident_sb[:], m,
                                         start=False, stop=True)
                        nc.tensor.matmul(sp_t[:, QBLK:], ident_sb[:], m,
                                         start=False, stop=True)
                    a_p = attn_pool.tile([P, 2 * QBLK], F32R, tag="ap",
                                         name="a_p")
                    nc.scalar.activation(a_p[:], sp_t[:], EXP)
                    # software pipeline: ctx of the previous k-block issues
                    # after this block's scores, so PE runs ahead of ACT.
                    if pend is not None:
                        emit_ctx(pend)
                    pend = (a_p, jj, ki == 0, ki == nkb - 1)
                emit_ctx(pend)

                # normalize ctx^T by 1/denominator (row 64), partition-
                # broadcast the reciprocal via a DRAM bounce.
                rec = small.tile([P, 2 * QBLK], F32, tag="rec")
                nc.vector.reciprocal(rec[DK:DA, 0:QBLK], C0[DK:DA])
                nc.vector.reciprocal(rec[DK:DA, QBLK:], C1[DK:DA])
                rec_dr = dram.tile([1, 2 * QBLK], F32, tag="rec_dr",
                                   name="rec_dr")
                nc.sync.dma_start(rec_dr[:], rec[DK:DA, :])
                rb_sb = small.tile([P, 2 * QBLK], F32, tag="rb_sb")
                nc.sync.dma_start(rb_sb[0:DK, :],
                                  rec_dr[:].to_broadcast((DK, 2 * QBLK)))
                ctx0_sb = small.tile([P, QBLK], F32R, tag="ctx0_sb")
                ctx1_sb = small.tile([P, QBLK], F32R, tag="ctx1_sb")
                nc.vector.tensor_mul(ctx0_sb[0:DK], C0[0:DK],
                                     rb_sb[0:DK, 0:QBLK])
                nc.vector.tensor_mul(ctx1_sb[0:DK], C1[0:DK],
                                     rb_sb[0:DK, QBLK:])
                assert QBLK % TPC == 0
                for sub in range(QBLK // TPC):
                    chunk = (b * T + qi * QBLK) // TPC + sub
                    nc.sync.dma_start(a2a_in[chunk, 0:DK],
                                      ctx0_sb[0:DK, ts(sub, TPC)])
                    nc.sync.dma_start(a2a_in[chunk, DK:P],
                                      ctx1_sb[0:DK, ts(sub, TPC)])

            # ---- phase plan: batch-0 proj -> batch-0 attention while
            # batch-1 proj/v_nat fill PE gaps -> batch-1 attention --------
            half_pairs = NPAIR // B  # proj pairs per batch
            for i in range(half_pairs):
                proj_pair(i)
            for j in range(NTOK // P // B):
                v_nat_block(j)

            # wide constants for the tail, loaded mid-kernel so they don't
            # fight the startup DMA burst
            wo_sb = consts.tile([P, KO, D], F32R, tag="wo")
            bo_sb = consts.tile([P, D], F32, tag="bo")

            late = []
            for i in range(half_pairs, NPAIR):
                late.append(lambda i=i: proj_pair(i))
            late.append(lambda: nc.sync.dma_start(
                wo_sb[:], woT_d.ap().rearrange("(ko p) m -> p ko m", p=P)))
            late.append(lambda: nc.sync.dma_start(
                bo_sb[:], bo_d.ap()[None, :].to_broadcast((P, D))))
            for j0 in range(NTOK // P // B, NTOK // P, 4):
                late.append(lambda j0=j0: [v_nat_block(j)
                                           for j in range(j0, j0 + 4)])

            for qi in range(NQB):
                attention_qblock(0, qi)
                # interleave deferred batch-1 work into batch-0 attention
                nlate = max(1, (len(late) + NQB - 1 - qi) // (NQB - qi))
                for _ in range(min(nlate, len(late))):
                    late.pop(0)()
            while late:
                late.pop(0)()
            for qi in range(NQB):
                attention_qblock(1, qi)

            # ---- AllToAll over token slices -----------------------------
            nc.gpsimd.collective_compute(
                "AllToAll",
                mybir.AluOpType.bypass,
                replica_groups=[list(range(NCORES))],
                ins=[a2a_in[:].opt()],
                outs=[a2a_out[:].opt()],
            )

            # ---- output projection (token-sharded) ----------------------
            ctxg = outg.tile([P, KO, TPC], F32R, tag="ctxg")
            nc.sync.dma_start(ctxg[:], a2a_out[:].rearrange("j p t -> p j t"))
            for tb in range(TB):
                po = psA.tile([P, 2 * QBLK], F32, tag="sp", name="po")
                for ko in range(KO):
                    nc.tensor.matmul(po[:, 0:QBLK], ctxg[:, ko, ts(tb, P)],
                                     wo_sb[:, ko, 0:QBLK],
                                     start=(ko == 0), stop=(ko == KO - 1))
                    nc.tensor.matmul(po[:, QBLK:], ctxg[:, ko, ts(tb, P)],
                                     wo_sb[:, ko, QBLK:],
                                     start=(ko == 0), stop=(ko == KO - 1))
                o_sb = outp.tile([P, D], F32, tag="o_sb")
                nc.vector.tensor_add(o_sb[:], po[:], bo_sb[:])
                nc.sync.dma_start(out_d.ap()[ts(tb, P), :], o_sb[:])

    nc.compile()
    return nc


_NC_CACHE = {}


def _get_nc(B, T):
    key = (B, T)
    if key not in _NC_CACHE:
        _NC_CACHE[key] = build_nc(B, T)
    return _NC_CACHE[key]


def make_in_maps(x, Wq, bq, Wk, bk, Wv, bv, Wo, bo):
    B, T, _ = x.shape
    NTOK = B * T
    NSLAB = NTOK // QBLK
    KO = D // P
    x = np.asarray(x, np.float32)
    # [D, NTOK] -> [p, slab, ko, t]: one contiguous 16KB DMA descriptor per
    # partition per slab.
    xT = x.reshape(NTOK, D).T  # [D, NTOK]
    xT_t = np.ascontiguousarray(
        xT.reshape(KO, P, NSLAB, QBLK).transpose(1, 2, 0, 3))
    woT = np.ascontiguousarray(np.asarray(Wo, np.float32).T)
    bo = np.asarray(bo, np.float32)
    # causal staircase bias: 0 where allowed (c >= kk + (QBLK-KBLK)),
    # -300 where masked; accumulated into scores via an identity-stationary
    # matmul so exp() of masked entries underflows to zero.
    keep = (np.arange(2 * QBLK - KBLK)[None, :]
            >= (np.arange(P)[:, None] + (QBLK - KBLK)))
    mask = np.where(keep, 0.0, -300.0).astype(np.float32)
    ident = np.eye(P, dtype=np.float32)
    ones = np.ones((P, P), np.float32)
    in_maps = []
    for c in range(NCORES):
        sl = slice(DSL * c, DSL * (c + 1))
        in_maps.append({
            "xT": xT_t,
            "wqT": np.ascontiguousarray(np.asarray(Wq, np.float32)[sl].T) * 0.125,
            "wkT": np.ascontiguousarray(np.asarray(Wk, np.float32)[sl].T),
            "wvT": np.ascontiguousarray(np.asarray(Wv, np.float32)[sl].T),
            "woT": woT,
            "bq": (np.asarray(bq, np.float32)[sl] * 0.125).reshape(DSL, 1),
            "bk": np.asarray(bk, np.float32)[sl].reshape(DSL, 1),
            "bv": np.asarray(bv, np.float32)[sl].reshape(DSL, 1),
            "bo": bo,
            "mask": mask,
            "ident": ident,
            "ones": ones,
        })
    return in_maps


LAST_RESULTS = None


def kernel(x, Wq, bq, Wk, bk, Wv, bv, Wo, bo, trace=False, trace_cores=None):
    global LAST_RESULTS
    B, T, _ = x.shape
    nc = _get_nc(B, T)
    in_maps = make_in_maps(x, Wq, bq, Wk, bk, Wv, bv, Wo, bo)
    kw = {}
    if trace:
        kw = dict(trace=True, trace_cores=trace_cores)
    res = bass_utils.run_bass_kernel_spmd(nc, in_maps,
                                          core_ids=list(range(NCORES)), **kw)
    LAST_RESULTS = res
    out = np.concatenate([res.results[c]["out"] for c in range(NCORES)], axis=0)
    return out.reshape(B, T, D)
